# revision 29
# baseline (speedup 1.0000x reference)
"""Trainium2 Bass kernel for nn_Autoregression — fp8 DoubleRow, lean head.

Math: log_prob[b,k,t] = -0.5*(C*log(2pi) + logdet(Sigma_k)
+ ||L_k^{-1}(conv(x,W_k)+b_k)||^2).  Fold L^{-1} into the conv (W2, b2);
mahal = sum_c (es_c + b2_c)^2 with es = conv9(x; W2) (bias applied in
the squaring stage, NOT in the matmul).

Device layout (per core, T sharded 8 ways, 8-sample left halo):
es computed as [kc, t] PSUM tiles (kc = 2 states x 64 ch per block, 8
blocks) so the channel reduction runs on the PE.  Conv contraction
576 = 64ci x 9taps done per block per 512-t block as 2 fp8 DoubleRow
matmuls (taps 0-7, 2x128-row k-tiles each, x moving with a stride-2
overlapped AP) + 1 plain fp8 matmul (tap 8: 64 weight rows + 64 zero
rows; rhs is xin shifted +8, upper-copy partitions are multiplied by
zero weights).  This removes the old xed tensor (ones row / dead zone)
entirely: 1.1MB less DMA per core and one fewer input tensor.

Squares add the folded bias b2 on the fly: groups 0-2 on Act
(sq = Square(es + b2), bias is a per-partition [128,1] AP, one instr
per h since the two halves need different biases); group 3 adds b2 on
DVE (tensor_scalar add -> bf16) and squares on GpSimd (DVE
TensorTensor cannot read PSUM on both ports).

4 DoubleRow mask matmuls reduce 64-channel groups -> mahal PSUM
[16, t]; DVE tensor_scalar applies -0.5*x+bias -> out SBUF f32;
sync-ring DMA out.  PE waits are kept single-semaphore via observer
matmuls (baseline trick).

Head: warmup matmuls run on a memset SBUF tile (no DMA dependency) so
the PE ramps from ~0.3us and HAM is warm when real data lands; the
critical first-block DMAs (biases, xin chunk 0, g3 weights) are
front-loaded on the sync queue, which is the first DMA ring to come
up.
"""

import math
import os

import numpy as np
import ml_dtypes

import concourse.bass as bass
import concourse.bacc as bacc_mod
import concourse.mybir as mybir
import concourse.tile as tile
from concourse.bass_utils import run_bass_kernel_spmd
from concourse.tile_rust import add_dep_helper
import bass_rust

K = 16
C = 64
T = 65536
AR = 8
NCORES = 8
TLOC = T // NCORES
TB = 512                    # t per block-iteration
NB = 8                      # kc blocks (2 states x 64ch each)
NG = 4                      # es groups per t-block (2 kc blocks each)
WARMN = 54                  # DMA-independent PE warmup matmuls
PIN_OBS = False             # pin observers behind the latest real MM

FP8 = mybir.dt.float8e4
_FP8_NP = ml_dtypes.float8_e4m3

_CACHE: dict = {}


def _chunks(tloc):
    """xin DMA chunks: [0, 520), then 4-t-block strides with halo."""
    ntb = tloc // TB
    bnds = [0, 1, 2] + [2 + 4 * i for i in range(1, (ntb + 2) // 4)] + [ntb]
    bnds = sorted(set(b for b in bnds if b <= ntb))
    out = []
    for a, b in zip(bnds[:-1], bnds[1:]):
        lo = a * TB
        hi = (b - 1) * TB + TB + AR  # last col read: (b-1)*TB + 511 + 8
        out.append((a, lo, min(hi, tloc + AR + 1)))
    return out


def _build_program(tloc=TLOC):
    nc = bacc_mod.Bacc()
    f32 = mybir.dt.float32
    ntb = tloc // TB

    xin = nc.declare_dram_parameter("xin", [128, tloc + AR + 1], FP8, isOutput=False)
    wts = nc.declare_dram_parameter("wts", [128, 5 * NB, 128], FP8, isOutput=False)
    NW = 5 * NB + 1             # w slices + shared zero ktile (memset on device)
    maskd = nc.declare_dram_parameter("maskd", [128, 8, K], FP8, isOutput=False)
    biasd = nc.declare_dram_parameter("biasd", [K, 1], f32, isOutput=False)
    brow = nc.declare_dram_parameter("brow", [128, NB], f32, isOutput=False)
    out = nc.declare_dram_parameter("out", [K, tloc], f32, isOutput=True)

    chunks = _chunks(tloc)

    with tile.TileContext(nc) as tc:
        with (
            tc.tile_pool(name="singles", bufs=1) as singles,
            tc.tile_pool(name="sqpool", bufs=3) as sqpool,
            tc.tile_pool(name="esbpool", bufs=2) as esbpool,
            tc.tile_pool(name="es_ps", bufs=6, space="PSUM") as es_ps,
            tc.tile_pool(name="m_ps", bufs=1, space="PSUM") as m_ps,
            tc.tile_pool(name="obs_ps", bufs=1, space="PSUM") as obs_ps,
        ):
            # --- observer machinery (single-sem matmul waits) ---
            scratch = obs_ps.tile([128, 128], f32)
            pending = []
            last_mm = [None]    # pin observers behind the latest real MM so
                                # the scheduler can't hoist them (a hoisted
                                # observer stalls the PE on a late DMA)

            def pe_observe(col):
                i = nc.tensor.matmul(
                    scratch[0:2, 0:2], col, col, start=True, stop=True
                )
                if PIN_OBS and last_mm[0] is not None:
                    add_dep_helper(i.ins, last_mm[0].ins, sync=False)
                pending.append(i)

            def _flush(i):
                while pending:
                    add_dep_helper(i.ins, pending.pop().ins, sync=False)
                return i

            # --- SBUF tiles ---
            w_sb = singles.tile([128, NW, 128], FP8)
            mask_sb = singles.tile([128, 8, K], FP8)
            bias_sb = singles.tile([K, 1], f32)
            brow_sb = singles.tile([128, NB], f32)
            dummy_sb = singles.tile([K, 1], f32)
            dummy2_sb = singles.tile([128, 1], f32)
            warm_sb = singles.tile([128, 128], FP8)
            xin_sb = singles.tile([128, tloc + AR + 1], FP8)
            out_sb = singles.tile([K, tloc], f32)

            # --- PE warmup: memset on GpSimd (earliest-executing engine),
            # so the PE ramps HAM before the first data lands.
            nc.gpsimd.memset(warm_sb, 0.0)
            # shared zero ktile for the tap-8 DoubleRow matmuls
            nc.gpsimd.memset(w_sb[:, 5 * NB, :], 0.0)
            for _ in range(WARMN):
                nc.tensor.matmul(
                    scratch[0:128, 0:128], warm_sb, warm_sb, start=True, stop=True
                )

            # --- input DMAs.  Issue cost is ~0.6-1.2us per dma_start on the
            # issuing engine's sequencer, and the Scalar/GpSimd queues come
            # alive ~2us before Sync — so the critical first-block transfers
            # are spread across engines: Scalar gets xin chunk 0 + g3
            # weights + mask, GpSimd gets the remaining weights, Vector the
            # tiny biases, Sync the later xin chunks (it also runs the out
            # ring).
            a0, lo0, hi0 = chunks[0]
            nc.gpsimd.dma_start(out=xin_sb[:, lo0:hi0], in_=xin[:, lo0:hi0])
            nc.gpsimd.dma_start(out=w_sb[:, 30:40, :], in_=wts[:, 30:40, :])
            nc.gpsimd.dma_start(out=brow_sb, in_=brow[:, :])
            nc.gpsimd.dma_start(out=bias_sb, in_=biasd[:, :])
            nc.scalar.dma_start(out=w_sb[:, 0:10, :], in_=wts[:, 0:10, :])
            nc.sync.dma_start(out=w_sb[:, 10:20, :], in_=wts[:, 10:20, :])
            nc.sync.dma_start(out=w_sb[:, 20:30, :], in_=wts[:, 20:30, :])
            nc.scalar.dma_start(out=mask_sb, in_=maskd[:, :, :])
            a1, lo1, hi1 = chunks[1]
            nc.scalar.dma_start(out=xin_sb[:, lo1:hi1], in_=xin[:, lo1:hi1])
            for a, lo, hi in chunks[2:]:
                nc.sync.dma_start(out=xin_sb[:, lo:hi], in_=xin[:, lo:hi])

            # DVE: touch biases first (absorbs their DMA sems on DVE's
            # in-order stream before the first dependent op needs them)
            nc.vector.tensor_copy(dummy_sb, bias_sb)
            nc.vector.tensor_copy(dummy2_sb, brow_sb[:, 0:1])

            chunk_starts = {a: i for i, (a, lo, hi) in enumerate(chunks)}

            def conv_rhs(tb, s):
                """moving x AP for step s."""
                off = tb * TB
                ap = xin_sb[:, off + 4 * s : off + 4 * s + TB].copy()
                p = ap.ap[0]
                if s < 2:
                    # [128(p), 2(ktile), TB] overlapped stride-2 view
                    ap.ap = bass_rust.VecI64Pair([[p[0], p[1]], [2, 2], [1, TB]])
                else:
                    # tap 8: ktile1 streams the same cols into the shared
                    # zero-weight slice (stride-0 ktile dim)
                    ap.ap = bass_rust.VecI64Pair([[p[0], p[1]], [0, 2], [1, TB]])
                return ap

            def conv_lhsT(b, s):
                if s < 2:
                    return w_sb[:, 5 * b + 2 * s : 5 * b + 2 * s + 2, :]
                # ktile0 = tap8 slice 5b+4, ktile1 = shared zero slice 5*NB
                ap = w_sb[:, 5 * b + 4 : 5 * b + 6, :].copy()
                p = ap.ap[0]
                ap.ap = bass_rust.VecI64Pair(
                    [[p[0], p[1]], [(5 * NB - 5 * b - 4) * 128, 2], [1, 128]]
                )
                return ap

            DR = mybir.MatmulPerfMode.DoubleRow
            mlast = {}

            def emit_masks(tb):
                m = m_ps.tile([K, TB], f32, name="m", tag="m")
                sq = sqtiles[tb]
                for p in range(4):
                    i = nc.tensor.matmul(
                        m[:, :],
                        mask_sb[:, 2 * p : 2 * p + 2, :],
                        sq[:, 2 * p : 2 * p + 2, :],
                        start=(p == 0),
                        stop=(p == 3),
                        perf_mode=DR,
                    )
                    last_mm[0] = i
                    if p == 0:
                        _flush(i)
                mlast[tb] = m

            def emit_affine(tb):
                nc.vector.tensor_scalar(
                    out=out_sb[:, tb * TB : (tb + 1) * TB],
                    in0=mlast.pop(tb)[:, :],
                    scalar1=-0.5,
                    scalar2=bias_sb,
                    op0=mybir.AluOpType.mult,
                    op1=mybir.AluOpType.add,
                )

            sqtiles = {}
            for tb in range(ntb):
                sq = sqpool.tile([128, NB, TB], FP8, name="sq", tag="sq")
                sqtiles[tb] = sq
                gorder = (3, 0, 1, 2)
                for gi, g in enumerate(gorder):
                    if gi == 0:
                        if tb in chunk_starts:
                            off = tb * TB
                            pe_observe(xin_sb[:, off : off + 2])
                        if tb == 0:
                            pe_observe(w_sb[:, 5 * NB, 0:2])
                    esh = []
                    for h in range(2):
                        b = 2 * g + h
                        es = es_ps.tile([128, TB], f32, name="es", tag="es")
                        esh.append(es)
                        if tb == 0:
                            pe_observe(w_sb[:, 5 * b, 0:2])
                        for s in range(3):
                            i = nc.tensor.matmul(
                                es[:, :],
                                conv_lhsT(b, s),
                                conv_rhs(tb, s),
                                start=(s == 0),
                                stop=(s == 2),
                                perf_mode=DR,
                            )
                            last_mm[0] = i
                            if s == 0:
                                _flush(i)
                    with nc.allow_low_precision(
                        reason="squares quantized to fp8; validated host-side "
                        "(rel err ~1.4e-2 vs 2e-2 budget)"
                    ):
                        if g < 3:
                            for h in range(2):
                                nc.scalar.activation(
                                    sq[:, 2 * g + h, :],
                                    esh[h][:, :],
                                    mybir.ActivationFunctionType.Square,
                                    bias=brow_sb[:, 2 * g + h : 2 * g + h + 1],
                                )
                        else:
                            # DVE bias-add frees the PSUM es quickly; the
                            # square runs out-of-band on GpSimd from SBUF
                            esb = esbpool.tile(
                                [128, 2, TB], mybir.dt.bfloat16, name="esb", tag="esb"
                            )
                            for h in range(2):
                                nc.vector.tensor_scalar(
                                    out=esb[:, h, :],
                                    in0=esh[h][:, :],
                                    scalar1=brow_sb[:, 2 * g + h : 2 * g + h + 1],
                                    scalar2=None,
                                    op0=mybir.AluOpType.add,
                                )
                            nc.gpsimd.tensor_tensor(
                                sq[:, 2 * g : 2 * g + 2, :], esb, esb,
                                mybir.AluOpType.mult
                            )
                    if gi == 1 and tb > 0:
                        if tb == 1:
                            pe_observe(mask_sb[:, 0, 0:2])
                        # m pool is double-buffered: the WAR against the
                        # affine of tb-3 is covered transitively, no
                        # observer needed
                        emit_masks(tb - 1)
                    if gi == 2 and tb > 0:
                        emit_affine(tb - 1)
                    if gi == 3 and tb > 0:
                        t0 = (tb - 1) * TB
                        nc.sync.dma_start(
                            out=out[:, t0 : t0 + TB], in_=out_sb[:, t0 : t0 + TB]
                        )
            emit_masks(ntb - 1)
            emit_affine(ntb - 1)
            t0 = (ntb - 1) * TB
            nc.sync.dma_start(out=out[:, t0 : t0 + TB], in_=out_sb[:, t0 : t0 + TB])
    nc.compile()
    return nc


def _prep_host(W, b, Sigma):
    """Fold L^{-1} into conv weights; pack fp8 tiles + constants."""
    W64 = W.astype(np.float64)
    b64 = b.astype(np.float64)
    S64 = Sigma.astype(np.float64)
    L = np.linalg.cholesky(S64)
    Li = np.linalg.inv(L)
    logdet = 2.0 * np.sum(np.log(np.diagonal(L, axis1=1, axis2=2)), axis=1)
    W2 = np.einsum("kdc,kcij->kdij", Li, W64)   # [K, d, ci, 9]
    b2 = np.einsum("kdc,kc->kd", Li, b64)       # [K, d]

    W2q = W2.astype(np.float32).astype(_FP8_NP).astype(np.float32)

    # w_np[r, 5b+2s+i, m]: m = 64*(k-2b) + d
    #   s<2: = W2[2b + m//64, m%64, r%64, 4s+2i + r//64]
    #   s=2 (single slice 5b+4): r<64 -> W2[.., r, 8]; r>=64 -> 0
    w_np = np.zeros((128, 5 * NB, 128), np.float32)
    Wb = W2q.reshape(NB, 2, C, C, 9)            # [b, kin2, d, ci, j]
    for b_ in range(NB):
        for s in range(2):
            for i in range(2):
                for par in range(2):
                    j = 4 * s + 2 * i + par
                    blk = Wb[b_, :, :, :, j]    # [kin2, d, ci]
                    w_np[par * C : par * C + C, 5 * b_ + 2 * s + i, :] = (
                        blk.transpose(2, 0, 1).reshape(C, 128)
                    )
        w_np[0:C, 5 * b_ + 4, :] = (
            Wb[b_, :, :, :, 8].transpose(2, 0, 1).reshape(C, 128)
        )

    mask_np = np.zeros((128, 8, K), np.float32)
    r = np.arange(128)
    for p in range(4):
        for i in range(2):
            mask_np[r, 2 * p + i, 4 * p + 2 * i + r // C] = 1.0

    # per-block folded bias rows (f32, added in the squaring stage)
    brow_np = b2.astype(np.float32).reshape(NB, 128).T.copy()  # [128, NB]

    const = C * math.log(2.0 * math.pi) + logdet
    bias_np = (-0.5 * const).astype(np.float32).reshape(K, 1)
    return w_np.astype(_FP8_NP), mask_np.astype(_FP8_NP), bias_np, brow_np


def _make_in_maps(x, w_np, mask_np, bias_np, brow_np, tloc=TLOC, ncores=NCORES):
    xq = np.asarray(x, np.float32)[0].astype(_FP8_NP).astype(np.float32)
    xpad = np.pad(xq, ((0, 0), (AR, 2)))        # [C, AR+T+2]
    in_maps = []
    for i in range(ncores):
        lo = xpad[:, tloc * i : tloc * i + tloc + AR + 1]
        hi = xpad[:, tloc * i + 1 : tloc * i + tloc + AR + 2]
        in_maps.append(
            {
                "xin": np.ascontiguousarray(
                    np.concatenate([lo, hi], axis=0).astype(_FP8_NP)
                ),
                "wts": w_np,
                "maskd": mask_np,
                "biasd": bias_np,
                "brow": brow_np,
            }
        )
    return in_maps


def _run(x, W, b, Sigma, trace=False):
    if "nc" not in _CACHE:
        _CACHE["nc"] = _build_program()
    nc = _CACHE["nc"]
    w_np, mask_np, bias_np, brow_np = _prep_host(
        np.asarray(W, np.float32), np.asarray(b, np.float32),
        np.asarray(Sigma, np.float32),
    )
    in_maps = _make_in_maps(np.asarray(x, np.float32), w_np, mask_np, bias_np, brow_np)
    res = run_bass_kernel_spmd(
        nc, in_maps, core_ids=list(range(NCORES)), trace=trace
    )
    outs = [res.results[i]["out"] for i in range(NCORES)]
    full = np.concatenate(outs, axis=1)[None]   # [1, K, T]
    return full.astype(np.float32), res


def kernel(x, W, b, Sigma):
    out, _ = _run(x, W, b, Sigma, trace=bool(int(os.environ.get("BASS_TRACE", "0"))))
    return out


# revision 30
# speedup vs baseline: 1.0370x; 1.0370x over previous
"""Trainium2 Bass kernel for nn_Autoregression — fp8 DoubleRow, lean head.

Math: log_prob[b,k,t] = -0.5*(C*log(2pi) + logdet(Sigma_k)
+ ||L_k^{-1}(conv(x,W_k)+b_k)||^2).  Fold L^{-1} into the conv (W2, b2);
mahal = sum_c (es_c + b2_c)^2 with es = conv9(x; W2) (bias applied in
the squaring stage, NOT in the matmul).

Device layout (per core, T sharded 8 ways, 8-sample left halo):
es computed as [kc, t] PSUM tiles (kc = 2 states x 64 ch per block, 8
blocks) so the channel reduction runs on the PE.  Conv contraction
576 = 64ci x 9taps done per block per 512-t block as 2 fp8 DoubleRow
matmuls (taps 0-7, 2x128-row k-tiles each, x moving with a stride-2
overlapped AP) + 1 plain fp8 matmul (tap 8: 64 weight rows + 64 zero
rows; rhs is xin shifted +8, upper-copy partitions are multiplied by
zero weights).  This removes the old xed tensor (ones row / dead zone)
entirely: 1.1MB less DMA per core and one fewer input tensor.

Squares add the folded bias b2 on the fly: groups 0-2 on Act
(sq = Square(es + b2), bias is a per-partition [128,1] AP, one instr
per h since the two halves need different biases); group 3 adds b2 on
DVE (tensor_scalar add -> bf16) and squares on GpSimd (DVE
TensorTensor cannot read PSUM on both ports).

4 DoubleRow mask matmuls reduce 64-channel groups -> mahal PSUM
[16, t]; DVE tensor_scalar applies -0.5*x+bias -> out SBUF f32;
sync-ring DMA out.  PE waits are kept single-semaphore via observer
matmuls (baseline trick).

Head: warmup matmuls run on a memset SBUF tile (no DMA dependency) so
the PE ramps from ~0.3us and HAM is warm when real data lands; the
critical first-block DMAs (biases, xin chunk 0, g3 weights) are
front-loaded on the sync queue, which is the first DMA ring to come
up.
"""

import math
import os

import numpy as np
import ml_dtypes

import concourse.bass as bass
import concourse.bacc as bacc_mod
import concourse.mybir as mybir
import concourse.tile as tile
from concourse.bass_utils import run_bass_kernel_spmd
from concourse.tile_rust import add_dep_helper
import bass_rust

K = 16
C = 64
T = 65536
AR = 8
NCORES = 8
TLOC = T // NCORES
TB = 512                    # t per block-iteration
NB = 8                      # kc blocks (2 states x 64ch each)
NG = 4                      # es groups per t-block (2 kc blocks each)
WARMN = 54                  # DMA-independent PE warmup matmuls
PIN_OBS = False             # pin observers behind the latest real MM

FP8 = mybir.dt.float8e4
_FP8_NP = ml_dtypes.float8_e4m3

_CACHE: dict = {}


def _chunks(tloc):
    """xin DMA chunks: [0, 520), then 4-t-block strides with halo."""
    ntb = tloc // TB
    bnds = [0, 1, 2] + [2 + 4 * i for i in range(1, (ntb + 2) // 4)] + [ntb]
    bnds = sorted(set(b for b in bnds if b <= ntb))
    out = []
    for a, b in zip(bnds[:-1], bnds[1:]):
        lo = a * TB
        hi = (b - 1) * TB + TB + AR  # last col read: (b-1)*TB + 511 + 8
        out.append((a, lo, min(hi, tloc + AR + 1)))
    return out


def _build_program(tloc=TLOC):
    nc = bacc_mod.Bacc()
    f32 = mybir.dt.float32
    ntb = tloc // TB

    xin = nc.declare_dram_parameter("xin", [128, tloc + AR + 1], FP8, isOutput=False)
    wts = nc.declare_dram_parameter("wts", [128, 5 * NB, 128], FP8, isOutput=False)
    NW = 5 * NB + 1             # w slices + shared zero ktile (memset on device)
    maskd = nc.declare_dram_parameter("maskd", [128, 8, K], FP8, isOutput=False)
    biasd = nc.declare_dram_parameter("biasd", [K, 1], f32, isOutput=False)
    brow = nc.declare_dram_parameter("brow", [128, NB], f32, isOutput=False)
    out = nc.declare_dram_parameter("out", [K, tloc], f32, isOutput=True)

    chunks = _chunks(tloc)

    with tile.TileContext(nc) as tc:
        with (
            tc.tile_pool(name="singles", bufs=1) as singles,
            tc.tile_pool(name="sqpool", bufs=2) as sqpool,
            tc.tile_pool(name="esbpool", bufs=2) as esbpool,
            tc.tile_pool(name="es_ps", bufs=6, space="PSUM") as es_ps,
            tc.tile_pool(name="m_ps", bufs=1, space="PSUM") as m_ps,
            tc.tile_pool(name="obs_ps", bufs=1, space="PSUM") as obs_ps,
        ):
            # --- observer machinery (single-sem matmul waits) ---
            scratch = obs_ps.tile([128, 128], f32)
            pending = []
            last_mm = [None]    # pin observers behind the latest real MM so
                                # the scheduler can't hoist them (a hoisted
                                # observer stalls the PE on a late DMA)

            def pe_observe(col):
                i = nc.tensor.matmul(
                    scratch[0:2, 0:2], col, col, start=True, stop=True
                )
                if PIN_OBS and last_mm[0] is not None:
                    add_dep_helper(i.ins, last_mm[0].ins, sync=False)
                pending.append(i)

            def _flush(i):
                while pending:
                    add_dep_helper(i.ins, pending.pop().ins, sync=False)
                return i

            # --- SBUF tiles ---
            w_sb = singles.tile([128, NW, 128], FP8)
            mask_sb = singles.tile([128, 8, K], FP8)
            bias_sb = singles.tile([K, 1], f32)
            brow_sb = singles.tile([128, NB], f32)
            dummy_sb = singles.tile([K, 1], f32)
            dummy2_sb = singles.tile([128, 1], f32)
            warm_sb = singles.tile([128, 128], FP8)
            xin_sb = singles.tile([128, tloc + AR + 1], FP8)
            out_sb = singles.tile([K, tloc], f32)

            # --- PE warmup: memset on GpSimd (earliest-executing engine),
            # so the PE ramps HAM before the first data lands.
            nc.gpsimd.memset(warm_sb, 0.0)
            # shared zero ktile for the tap-8 DoubleRow matmuls
            nc.gpsimd.memset(w_sb[:, 5 * NB, :], 0.0)
            for _ in range(WARMN):
                nc.tensor.matmul(
                    scratch[0:128, 0:128], warm_sb, warm_sb, start=True, stop=True
                )

            # --- input DMAs.  Issue cost is ~0.6-1.2us per dma_start on the
            # issuing engine's sequencer, and the Scalar/GpSimd queues come
            # alive ~2us before Sync — so the critical first-block transfers
            # are spread across engines: Scalar gets xin chunk 0 + g3
            # weights + mask, GpSimd gets the remaining weights, Vector the
            # tiny biases, Sync the later xin chunks (it also runs the out
            # ring).
            a0, lo0, hi0 = chunks[0]
            nc.gpsimd.dma_start(out=xin_sb[:, lo0:hi0], in_=xin[:, lo0:hi0])
            nc.gpsimd.dma_start(out=w_sb[:, 30:40, :], in_=wts[:, 30:40, :])
            nc.gpsimd.dma_start(out=w_sb[:, 0:10, :], in_=wts[:, 0:10, :])
            nc.gpsimd.dma_start(out=brow_sb, in_=brow[:, :])
            nc.gpsimd.dma_start(out=bias_sb, in_=biasd[:, :])
            nc.sync.dma_start(out=w_sb[:, 10:20, :], in_=wts[:, 10:20, :])
            nc.sync.dma_start(out=w_sb[:, 20:30, :], in_=wts[:, 20:30, :])
            nc.scalar.dma_start(out=mask_sb, in_=maskd[:, :, :])
            a1, lo1, hi1 = chunks[1]
            nc.scalar.dma_start(out=xin_sb[:, lo1:hi1], in_=xin[:, lo1:hi1])
            for a, lo, hi in chunks[2:]:
                nc.sync.dma_start(out=xin_sb[:, lo:hi], in_=xin[:, lo:hi])

            # DVE: touch biases first (absorbs their DMA sems on DVE's
            # in-order stream before the first dependent op needs them)
            nc.vector.tensor_copy(dummy_sb, bias_sb)
            nc.vector.tensor_copy(dummy2_sb, brow_sb[:, 0:1])

            chunk_starts = {a: i for i, (a, lo, hi) in enumerate(chunks)}

            def conv_rhs(tb, s):
                """moving x AP for step s."""
                off = tb * TB
                ap = xin_sb[:, off + 4 * s : off + 4 * s + TB].copy()
                p = ap.ap[0]
                if s < 2:
                    # [128(p), 2(ktile), TB] overlapped stride-2 view
                    ap.ap = bass_rust.VecI64Pair([[p[0], p[1]], [2, 2], [1, TB]])
                else:
                    # tap 8: ktile1 streams the same cols into the shared
                    # zero-weight slice (stride-0 ktile dim)
                    ap.ap = bass_rust.VecI64Pair([[p[0], p[1]], [0, 2], [1, TB]])
                return ap

            def conv_lhsT(b, s):
                if s < 2:
                    return w_sb[:, 5 * b + 2 * s : 5 * b + 2 * s + 2, :]
                # ktile0 = tap8 slice 5b+4, ktile1 = shared zero slice 5*NB
                ap = w_sb[:, 5 * b + 4 : 5 * b + 6, :].copy()
                p = ap.ap[0]
                ap.ap = bass_rust.VecI64Pair(
                    [[p[0], p[1]], [(5 * NB - 5 * b - 4) * 128, 2], [1, 128]]
                )
                return ap

            DR = mybir.MatmulPerfMode.DoubleRow
            mlast = {}

            def emit_masks(tb):
                m = m_ps.tile([K, TB], f32, name="m", tag="m")
                sq = sqtiles[tb]
                for p in range(4):
                    i = nc.tensor.matmul(
                        m[:, :],
                        mask_sb[:, 2 * p : 2 * p + 2, :],
                        sq[:, 2 * p : 2 * p + 2, :],
                        start=(p == 0),
                        stop=(p == 3),
                        perf_mode=DR,
                    )
                    last_mm[0] = i
                    if p == 0:
                        _flush(i)
                mlast[tb] = m

            def emit_affine(tb):
                nc.vector.tensor_scalar(
                    out=out_sb[:, tb * TB : (tb + 1) * TB],
                    in0=mlast.pop(tb)[:, :],
                    scalar1=-0.5,
                    scalar2=bias_sb,
                    op0=mybir.AluOpType.mult,
                    op1=mybir.AluOpType.add,
                )

            sqtiles = {}
            for tb in range(ntb):
                sq = sqpool.tile([128, NB, TB], FP8, name="sq", tag="sq")
                sqtiles[tb] = sq
                gorder = (3, 0, 1, 2)
                for gi, g in enumerate(gorder):
                    if gi == 0:
                        if tb in chunk_starts:
                            off = tb * TB
                            pe_observe(xin_sb[:, off : off + 2])
                        if tb == 0:
                            pe_observe(w_sb[:, 5 * NB, 0:2])
                    esh = []
                    for h in range(2):
                        b = 2 * g + h
                        es = es_ps.tile([128, TB], f32, name="es", tag="es")
                        esh.append(es)
                        if tb == 0:
                            pe_observe(w_sb[:, 5 * b, 0:2])
                        for s in range(3):
                            i = nc.tensor.matmul(
                                es[:, :],
                                conv_lhsT(b, s),
                                conv_rhs(tb, s),
                                start=(s == 0),
                                stop=(s == 2),
                                perf_mode=DR,
                            )
                            last_mm[0] = i
                            if s == 0:
                                _flush(i)
                    with nc.allow_low_precision(
                        reason="squares quantized to fp8; validated host-side "
                        "(rel err ~1.4e-2 vs 2e-2 budget)"
                    ):
                        if g < 3:
                            for h in range(2):
                                nc.scalar.activation(
                                    sq[:, 2 * g + h, :],
                                    esh[h][:, :],
                                    mybir.ActivationFunctionType.Square,
                                    bias=brow_sb[:, 2 * g + h : 2 * g + h + 1],
                                )
                        else:
                            # DVE bias-add frees the PSUM es quickly; the
                            # square runs out-of-band on GpSimd from SBUF
                            esb = esbpool.tile(
                                [128, 2, TB], mybir.dt.bfloat16, name="esb", tag="esb"
                            )
                            for h in range(2):
                                nc.vector.tensor_scalar(
                                    out=esb[:, h, :],
                                    in0=esh[h][:, :],
                                    scalar1=brow_sb[:, 2 * g + h : 2 * g + h + 1],
                                    scalar2=None,
                                    op0=mybir.AluOpType.add,
                                )
                            nc.gpsimd.tensor_tensor(
                                sq[:, 2 * g : 2 * g + 2, :], esb, esb,
                                mybir.AluOpType.mult
                            )
                    if gi == 1 and tb > 0:
                        if tb == 1:
                            pe_observe(mask_sb[:, 0, 0:2])
                        # m pool is double-buffered: the WAR against the
                        # affine of tb-3 is covered transitively, no
                        # observer needed
                        emit_masks(tb - 1)
                    if gi == 2 and tb > 0:
                        emit_affine(tb - 1)
                    if gi == 3 and tb > 0:
                        t0 = (tb - 1) * TB
                        nc.sync.dma_start(
                            out=out[:, t0 : t0 + TB], in_=out_sb[:, t0 : t0 + TB]
                        )
            emit_masks(ntb - 1)
            emit_affine(ntb - 1)
            t0 = (ntb - 1) * TB
            nc.sync.dma_start(out=out[:, t0 : t0 + TB], in_=out_sb[:, t0 : t0 + TB])
    nc.compile()
    return nc


def _prep_host(W, b, Sigma):
    """Fold L^{-1} into conv weights; pack fp8 tiles + constants."""
    W64 = W.astype(np.float64)
    b64 = b.astype(np.float64)
    S64 = Sigma.astype(np.float64)
    L = np.linalg.cholesky(S64)
    Li = np.linalg.inv(L)
    logdet = 2.0 * np.sum(np.log(np.diagonal(L, axis1=1, axis2=2)), axis=1)
    W2 = np.einsum("kdc,kcij->kdij", Li, W64)   # [K, d, ci, 9]
    b2 = np.einsum("kdc,kc->kd", Li, b64)       # [K, d]

    W2q = W2.astype(np.float32).astype(_FP8_NP).astype(np.float32)

    # w_np[r, 5b+2s+i, m]: m = 64*(k-2b) + d
    #   s<2: = W2[2b + m//64, m%64, r%64, 4s+2i + r//64]
    #   s=2 (single slice 5b+4): r<64 -> W2[.., r, 8]; r>=64 -> 0
    w_np = np.zeros((128, 5 * NB, 128), np.float32)
    Wb = W2q.reshape(NB, 2, C, C, 9)            # [b, kin2, d, ci, j]
    for b_ in range(NB):
        for s in range(2):
            for i in range(2):
                for par in range(2):
                    j = 4 * s + 2 * i + par
                    blk = Wb[b_, :, :, :, j]    # [kin2, d, ci]
                    w_np[par * C : par * C + C, 5 * b_ + 2 * s + i, :] = (
                        blk.transpose(2, 0, 1).reshape(C, 128)
                    )
        w_np[0:C, 5 * b_ + 4, :] = (
            Wb[b_, :, :, :, 8].transpose(2, 0, 1).reshape(C, 128)
        )

    mask_np = np.zeros((128, 8, K), np.float32)
    r = np.arange(128)
    for p in range(4):
        for i in range(2):
            mask_np[r, 2 * p + i, 4 * p + 2 * i + r // C] = 1.0

    # per-block folded bias rows (f32, added in the squaring stage)
    brow_np = b2.astype(np.float32).reshape(NB, 128).T.copy()  # [128, NB]

    const = C * math.log(2.0 * math.pi) + logdet
    bias_np = (-0.5 * const).astype(np.float32).reshape(K, 1)
    return w_np.astype(_FP8_NP), mask_np.astype(_FP8_NP), bias_np, brow_np


def _make_in_maps(x, w_np, mask_np, bias_np, brow_np, tloc=TLOC, ncores=NCORES):
    xq = np.asarray(x, np.float32)[0].astype(_FP8_NP).astype(np.float32)
    xpad = np.pad(xq, ((0, 0), (AR, 2)))        # [C, AR+T+2]
    in_maps = []
    for i in range(ncores):
        lo = xpad[:, tloc * i : tloc * i + tloc + AR + 1]
        hi = xpad[:, tloc * i + 1 : tloc * i + tloc + AR + 2]
        in_maps.append(
            {
                "xin": np.ascontiguousarray(
                    np.concatenate([lo, hi], axis=0).astype(_FP8_NP)
                ),
                "wts": w_np,
                "maskd": mask_np,
                "biasd": bias_np,
                "brow": brow_np,
            }
        )
    return in_maps


def _run(x, W, b, Sigma, trace=False):
    if "nc" not in _CACHE:
        _CACHE["nc"] = _build_program()
    nc = _CACHE["nc"]
    w_np, mask_np, bias_np, brow_np = _prep_host(
        np.asarray(W, np.float32), np.asarray(b, np.float32),
        np.asarray(Sigma, np.float32),
    )
    in_maps = _make_in_maps(np.asarray(x, np.float32), w_np, mask_np, bias_np, brow_np)
    res = run_bass_kernel_spmd(
        nc, in_maps, core_ids=list(range(NCORES)), trace=trace
    )
    outs = [res.results[i]["out"] for i in range(NCORES)]
    full = np.concatenate(outs, axis=1)[None]   # [1, K, T]
    return full.astype(np.float32), res


def kernel(x, W, b, Sigma):
    out, _ = _run(x, W, b, Sigma, trace=bool(int(os.environ.get("BASS_TRACE", "0"))))
    return out
